# revision 2
# baseline (speedup 1.0000x reference)
"""Trainium2 Bass kernel for nn_Conv_89713276879316.

Reference semantics (faithful channel bug): take ONLY the last channel of
image [32, 3, 512, 512], zero-pad by 7, cross-correlate with the 15x15
kernel, broadcast the [32, 1, 512, 512] result to all 3 channels.

Strategy (column-tiled multi-stream banded conv):
  The old baseline ran one 128-wide banded matmul stream: per 512-col
  moving pass it did 114 out rows x 15 taps = 1710 useful MACs/cycle of
  16384 (10%), needing 15 passes (one per kernel column dx) per tile ->
  ~64 us PE floor, ~104 us measured.

  This kernel packs 2x the taps per pass and 4 concurrent streams:
  - G=2 column-shift packing: each window's moving operand holds rows
    y0..y0+45 twice, at column shifts g=0 and g=1 (92 contract rows).
    One matmul pass then accumulates TWO kernel columns dx=2t+g at once,
    so 8 passes replace 15.
  - S=4 column tiling: four windows run as concurrent matmul streams on
    col-groups 0..3 of the PE array (tile_position=(0,32j)), each writing
    a 32-row output window into its own PSUM bank. Per cycle the array
    retires 4 windows x 32 outs x 30 taps = 3840 useful MACs (23%).
  PE floor: 64 windows/core / 4 streams * 8 passes * 512 cycles
  = 65,536 cycles ~= 27.3 us @ 2.4 GHz warm.

  Everything is bf16 (weights are exact 0/1 in bf16; image quantization
  gives ~4e-3 rel err, 5x inside the 2e-2 gate) with fp32 PSUM
  accumulation; outputs are drained to bf16 SBUF tiles (DVE/ACT
  alternating) and DMA'd out as bf16 (halves out traffic), host casts
  back to fp32. Host pre-builds the shifted-window layout in DRAM so the
  device DMA is plain contiguous slabs (6.2 MB in + 2.1 MB out per core
  ~= 23 us @ 358 GB/s, overlapped under the PE).
"""

import sys

import numpy as np

try:
    import concourse.bass as bass
except ImportError:  # pragma: no cover - fallback path inside the container
    sys.path.insert(0, "/opt/trn_rl_repo")
    import concourse.bass as bass

import ml_dtypes
from contextlib import ExitStack

import concourse.tile as tile
from concourse import bacc, mybir
from concourse.bass_utils import run_bass_kernel_spmd

N_CORES = 8
N_IMG = 32
C_IMG = 3
H = W = 512
KS = 15
PAD = KS // 2  # 7
HP = H + 2 * PAD  # 526
PER_CORE = N_IMG // N_CORES  # 4

G = 2  # column-shift replication factor
WIN = 32  # out rows per window (= one PSUM col-group slice)
ROWS = WIN + KS - 1  # 46 input rows per shift block
CTR = G * ROWS  # 92 contract rows
NPASS = (KS + G - 1) // G  # 8 matmul passes per window
NWIN_IMG = H // WIN  # 16 windows per image
NWIN = PER_CORE * NWIN_IMG  # 64 windows per core
NSTREAM = 4  # concurrent col-group streams
NSPAN = NWIN // NSTREAM  # 16 spans per core

F32 = mybir.dt.float32
BF16 = mybir.dt.bfloat16
NPBF16 = ml_dtypes.bfloat16

# tunables
SUP_BUFS = 12
OUT_BUFS = 8
PSUM_BUFS = 8
COPY_ENGINE = "alt"  # "dve" | "act" | "alt"

_CACHE = {}


def _build_nc(repeat=1, mode=None, loop=False):
    """Build the per-core Bass program (identical on all 8 cores).

    repeat > 1 re-runs the whole compute (same inputs/outputs) for
    dispatch-floor-free device timing; with loop=True the repetition is a
    Tile For_i loop (chunk-unrolled) instead of full unrolling.
    """
    nc = bacc.Bacc("TRN2", target_bir_lowering=False, debug=False)

    sup = nc.dram_tensor("sup", [NWIN, CTR, HP], BF16, kind="ExternalInput").ap()
    bands = nc.dram_tensor("bands", [128, NPASS * WIN], BF16, kind="ExternalInput").ap()
    out = nc.dram_tensor("out", [PER_CORE, H, W], BF16, kind="ExternalOutput").ap()

    with tile.TileContext(nc) as tc, ExitStack() as ctx:
        bands_pool = ctx.enter_context(tc.tile_pool(name="bands", bufs=1))
        sup_pool = ctx.enter_context(tc.tile_pool(name="sup", bufs=SUP_BUFS))
        psum_pool = ctx.enter_context(
            tc.tile_pool(name="psum", bufs=PSUM_BUFS, space="PSUM")
        )
        out_pool = ctx.enter_context(tc.tile_pool(name="outp", bufs=OUT_BUFS))

        bands_sb = bands_pool.tile([128, NPASS * WIN], BF16)
        nc.sync.dma_start(bands_sb[:], bands[:, :])

        cnt = 0

        def body(_iv=None):
            nonlocal cnt
            for s in range(NSPAN):
                units = []  # (w, sup_tile, psum_tile, j)
                for j in range(NSTREAM):
                    w = NSTREAM * s + j
                    t_sup = sup_pool.tile([CTR, HP], BF16, name="sup", tag="sup")
                    nc.sync.dma_start(t_sup[:], sup[w, :, :])
                    ps = psum_pool.tile([128, W], F32, name="ps", tag="ps")
                    units.append((w, t_sup, ps, j))

                for t in range(NPASS):
                    for (w, t_sup, ps, j) in units:
                        nc.tensor.matmul(
                            ps[WIN * j : WIN * (j + 1), :],
                            bands_sb[:CTR, WIN * t : WIN * (t + 1)],
                            t_sup[:, G * t : G * t + W],
                            start=(t == 0),
                            stop=(t == NPASS - 1),
                            tile_position=(0, WIN * j),
                        )

                for (w, t_sup, ps, j) in units:
                    ot = out_pool.tile([WIN, W], BF16, name="ot", tag="ot")
                    eng = COPY_ENGINE
                    if eng == "alt":
                        eng = "dve" if cnt % 2 == 0 else "act"
                    if eng == "dve":
                        nc.vector.tensor_copy(ot[:], ps[WIN * j : WIN * (j + 1), :])
                    else:
                        nc.scalar.copy(ot[:], ps[WIN * j : WIN * (j + 1), :])
                    cnt += 1
                    img, wi = divmod(w, NWIN_IMG)
                    nc.sync.dma_start(out[img, WIN * wi : WIN * (wi + 1), :], ot[:])

        if loop and repeat > 1:
            # unroll a few bodies per For_i iteration so the ~2us back-edge
            # barrier and lost cross-iteration overlap amortize away
            chunk = 4 if repeat % 4 == 0 else 1
            with tc.For_i(0, repeat // chunk, 1):
                for _u in range(chunk):
                    body()
        else:
            for _rep in range(repeat):
                body()

    nc.compile()
    return nc


def _prep_inputs(image: np.ndarray, kernel: np.ndarray):
    """Host-side prep: channel select, pad, shifted-window layout, bands."""
    ch = np.ascontiguousarray(image[:, -1, :, :]).astype(np.float32)  # [32,512,512]
    # width HP+1 so the g=1 shift can read one column past the padded image
    padded = np.zeros((N_IMG, HP, HP + G - 1), np.float32)
    padded[:, PAD : PAD + H, PAD : PAD + W] = ch
    pb = padded.astype(NPBF16)

    # sup[i, wi, g, u, x] = padded[i, 32*wi + u, x + g]
    sup = np.empty((N_IMG, NWIN_IMG, G, ROWS, HP), NPBF16)
    for wi in range(NWIN_IMG):
        for g in range(G):
            sup[:, wi, g, :, :] = pb[:, WIN * wi : WIN * wi + ROWS, g : g + HP]
    sup = sup.reshape(N_IMG, NWIN_IMG, CTR, HP)

    w = kernel.astype(np.float32)
    bands = np.zeros((128, NPASS, WIN), np.float32)
    for t in range(NPASS):
        for g in range(G):
            dx = G * t + g
            if dx >= KS:
                continue
            for m in range(WIN):
                bands[g * ROWS + m : g * ROWS + m + KS, t, m] = w[:, dx]
    bands_c = bands.reshape(128, NPASS * WIN).astype(NPBF16)
    return sup, bands_c


def make_in_maps(image: np.ndarray, kernel: np.ndarray):
    sup, bands_c = _prep_inputs(image, kernel)
    in_maps = []
    for c in range(N_CORES):
        s = slice(c * PER_CORE, (c + 1) * PER_CORE)
        in_maps.append(
            {
                "sup": np.ascontiguousarray(sup[s]).reshape(NWIN, CTR, HP),
                "bands": bands_c,
            }
        )
    return in_maps


def kernel(image: np.ndarray, kernel: np.ndarray) -> np.ndarray:
    in_maps = make_in_maps(image, kernel)

    key = "nc"
    if key not in _CACHE:
        _CACHE[key] = _build_nc()
    nc = _CACHE[key]

    res = run_bass_kernel_spmd(nc, in_maps, core_ids=list(range(N_CORES)))
    _CACHE["last_results"] = res

    full = np.concatenate(
        [res.results[c]["out"].astype(np.float32) for c in range(N_CORES)], axis=0
    )
    out = np.broadcast_to(full[:, None, :, :], (N_IMG, C_IMG, H, W))
    return np.ascontiguousarray(out)


# revision 24
# speedup vs baseline: 91.4173x; 91.4173x over previous
"""Trainium2 Bass kernel for nn_Conv_89713276879316.

Reference semantics (faithful channel bug): take ONLY the last channel of
image [32, 3, 512, 512], zero-pad by 7, cross-correlate with the 15x15
binary kernel, broadcast the [32, 1, 512, 512] result to all 3 channels.

Design: two concurrently-streaming banded matmuls (trn2's PE sustains two
moving-operand streams - it has two SBUF read ports), each a
[K=128, M=64, N=512] bf16 matmul over a 50-output-row window:

  - G=2 column-shift packing: the moving operand holds the window's 64
    input rows twice, at column shifts g=0 and g=1 (128 contract rows),
    so one pass accumulates TWO kernel columns dx=2t+g and 8 passes
    replace 15.  Stationary band matrix: S_t[g*64+u, m] = w[u-m, 2t+g].
  - The two windows run as column-tiled streams at tile_position (0,0) /
    (0,64) into separate PSUM banks, 4 banks double-buffered.
  - Per span (2 windows, 100 out rows): one 269 KB input DMA (gpsimd
    ring, off the sync ring that carries outputs), 16 matmuls, 2
    PSUM->SBUF drains (DVE/ACT alternating, fp32->bf16), one 128-row
    output DMA into a span-indexed DRAM slab; host reassembles + casts.

PE floor: 22 spans x 8 passes x 512 cols / 2 concurrent = 90,112 cycles
= 37.5 us warm @ 2.4 GHz; measured 41.2 us/body (vs 67 us for the prior
single-stream 114-row banded kernel, measured with the same harness).
bf16 is safe: weights are exact 0/1 in bf16, image quantization + bf16
output rounding give rel err 2.4e-3, 8x inside the 2e-2 gate.
"""


import sys

import numpy as np

try:
    import concourse.bass as bass
except ImportError:  # pragma: no cover
    sys.path.insert(0, "/opt/trn_rl_repo")
    import concourse.bass as bass

import ml_dtypes
from contextlib import ExitStack

import concourse.tile as tile
from concourse import bacc, mybir
from concourse.bass_utils import run_bass_kernel_spmd

N_CORES = 8
N_IMG = 32
C_IMG = 3
H = W = 512
KS = 15
PAD = KS // 2  # 7
HP = H + 2 * PAD  # 526
PER_CORE = N_IMG // N_CORES  # 4

G = 2  # column-shift packing
Y = 64  # input rows per shift block
CTR = G * Y  # 128 contract rows
WIN = Y - (KS - 1)  # 50 out rows per window
MCOL = 64  # PSUM col-group slice per stream
NPASS = (KS + G - 1) // G  # 8
NSTREAM = 2

# window starts per image: stride WIN, tail window flush against the end
_starts = list(range(0, H - WIN + 1, WIN))
if _starts[-1] != H - WIN:
    _starts.append(H - WIN)
NWIN_IMG = len(_starts)  # 11
NWIN = PER_CORE * NWIN_IMG  # 44
NSPAN = (NWIN + NSTREAM - 1) // NSTREAM  # 22

F32 = mybir.dt.float32
BF16 = mybir.dt.bfloat16
NPBF16 = ml_dtypes.bfloat16

SUP_BUFS = 6
OUT_BUFS = 6
PSUM_BUFS = 8
COPY_ENGINE = "alt"
DMA_IN_ENGINE = "gpsimd"
DMA_OUT_ENGINE = "sync"

_CACHE = {}


def _window_list():
    """Per-core ordered (img, y0) list, padded to a multiple of NSTREAM."""
    wl = [(i, y0) for i in range(PER_CORE) for y0 in _starts]
    while len(wl) % NSTREAM:
        wl.append(wl[-1])
    return wl


def _build_nc(repeat=1, mode=None, loop=False):
    nc = bacc.Bacc("TRN2", target_bir_lowering=False, debug=False)

    sup = nc.dram_tensor(
        "sup", [NSPAN, CTR, NSTREAM * HP], BF16, kind="ExternalInput"
    ).ap()
    bands = nc.dram_tensor(
        "bands", [CTR, NPASS * MCOL], BF16, kind="ExternalInput"
    ).ap()
    # span-indexed output: span s's two windows land at partition blocks
    # [0:50) and [64:114) of out[s]; host reassembles (garbage rows 50-63
    # and 114-128 are never read)
    out = nc.dram_tensor("out", [NSPAN, 128, W], BF16, kind="ExternalOutput").ap()

    wl = _window_list()

    with tile.TileContext(nc) as tc, ExitStack() as ctx:
        bands_pool = ctx.enter_context(tc.tile_pool(name="bands", bufs=1))
        sup_pool = ctx.enter_context(tc.tile_pool(name="sup", bufs=SUP_BUFS))
        psum_pool = ctx.enter_context(
            tc.tile_pool(name="psum", bufs=PSUM_BUFS, space="PSUM")
        )
        out_pool = ctx.enter_context(tc.tile_pool(name="outp", bufs=OUT_BUFS))

        bands_sb = bands_pool.tile([CTR, NPASS * MCOL], BF16)
        nc.sync.dma_start(bands_sb[:], bands[:, :])

        cnt = 0

        def body(_iv=None):
            nonlocal cnt
            for s in range(NSPAN):
                span_sup = sup_pool.tile(
                    [CTR, NSTREAM * HP], BF16, name="sup", tag="sup"
                )
                getattr(nc, DMA_IN_ENGINE).dma_start(span_sup[:], sup[s, :, :])
                units = []
                for j in range(NSTREAM):
                    ps = psum_pool.tile([128, W], F32, name="ps", tag="ps")
                    units.append((ps, j))

                for t in range(NPASS):
                    for (ps, j) in units:
                        nc.tensor.matmul(
                            ps[MCOL * j : MCOL * j + MCOL, :],
                            bands_sb[:, MCOL * t : MCOL * (t + 1)],
                            span_sup[:, j * HP + G * t : j * HP + G * t + W],
                            start=(t == 0),
                            stop=(t == NPASS - 1),
                            tile_position=(0, MCOL * j),
                        )

                # drains: stream j's valid rows sit at psum partitions
                # [64j, 64j+50); copy into the same partitions of one out
                # tile, then one DMA with a (2,50)-block partition AP to
                # contiguous DRAM rows (when both windows are y-adjacent)
                ot = out_pool.tile([128, W], BF16, name="ot", tag="ot")
                for (ps, j) in units:
                    eng = COPY_ENGINE
                    if eng == "alt":
                        eng = "dve" if cnt % 2 == 0 else "act"
                    # full 64-row halves so ot is fully written (rows
                    # [WIN,MCOL) hold defined-but-partial sums; host skips)
                    sl = slice(MCOL * j, MCOL * (j + 1))
                    if eng == "dve":
                        nc.vector.tensor_copy(ot[sl, :], ps[sl, :])
                    else:
                        nc.scalar.copy(ot[sl, :], ps[sl, :])
                    cnt += 1
                getattr(nc, DMA_OUT_ENGINE).dma_start(out[s, :, :], ot[:])

        if loop and repeat > 1:
            chunk = 4 if repeat % 4 == 0 else 1
            with tc.For_i(0, repeat // chunk, 1):
                for _u in range(chunk):
                    body()
        else:
            for _rep in range(repeat):
                body()

    nc.compile()
    return nc


def _prep_inputs(image: np.ndarray, kernel: np.ndarray):
    ch = np.ascontiguousarray(image[:, -1, :, :]).astype(np.float32)
    padded = np.zeros((N_IMG, HP, HP + G - 1), np.float32)
    padded[:, PAD : PAD + H, PAD : PAD + W] = ch
    pb = padded.astype(NPBF16)

    w = kernel.astype(np.float32)
    bands = np.zeros((CTR, NPASS, MCOL), np.float32)
    for t in range(NPASS):
        for g in range(G):
            dx = G * t + g
            if dx >= KS:
                continue
            for m in range(WIN):
                bands[g * Y + m : g * Y + m + KS, t, m] = w[:, dx]
    bands_c = bands.reshape(CTR, NPASS * MCOL).astype(NPBF16)
    return pb, bands_c


def make_in_maps(image: np.ndarray, kernel: np.ndarray):
    pb, bands_c = _prep_inputs(image, kernel)
    wl = _window_list()
    in_maps = []
    for c in range(N_CORES):
        sup = np.empty((NSPAN, CTR, NSTREAM, HP), NPBF16)
        for s in range(NSPAN):
            for j in range(NSTREAM):
                (i, y0) = wl[NSTREAM * s + j]
                img = pb[c * PER_CORE + i]
                for g in range(G):
                    sup[s, g * Y : (g + 1) * Y, j, :] = img[
                        y0 : y0 + Y, g : g + HP
                    ]
        in_maps.append(
            {
                "sup": np.ascontiguousarray(sup).reshape(NSPAN, CTR, NSTREAM * HP),
                "bands": bands_c,
            }
        )
    return in_maps


def assemble(core_outs):
    """[N_CORES x [NSPAN, 128, W] bf16] -> [N_IMG, H, W] fp32."""
    wl = _window_list()
    full = np.empty((N_IMG, H, W), np.float32)
    for c in range(N_CORES):
        o = core_outs[c].astype(np.float32)
        for s in range(NSPAN):
            for j in range(NSTREAM):
                (i, y0) = wl[NSTREAM * s + j]
                full[c * PER_CORE + i, y0 : y0 + WIN, :] = o[
                    s, MCOL * j : MCOL * j + WIN, :
                ]
    return full


def kernel(image: np.ndarray, kernel: np.ndarray) -> np.ndarray:
    in_maps = make_in_maps(image, kernel)
    key = "nc"
    if key not in _CACHE:
        _CACHE[key] = _build_nc()
    nc = _CACHE[key]
    res = run_bass_kernel_spmd(nc, in_maps, core_ids=list(range(N_CORES)))
    full = assemble([res.results[c]["out"] for c in range(N_CORES)])
    out = np.broadcast_to(full[:, None, :, :], (N_IMG, C_IMG, H, W))
    return np.ascontiguousarray(out)
